# revision 22
# baseline (speedup 1.0000x reference)
"""Causal multi-head attention (B=2, S=2048, D=2048, H=16) on 8 trn2 cores.

Tensor-parallel over heads: each core computes 2 heads end-to-end (QKV
projection, causal attention, output projection) and returns its partial
contribution to the output projection; the host sums the 8 partials and
adds the bias (the TP all-reduce).

Matmuls run in float32r (fp32 with 11-bit mantissa, full PE rate). The
compiler requires fp32r matmul inputs to be produced rounded, so DRAM
inputs are declared float32r and pre-rounded on the host; on-chip
producers (ACT/DVE copies) write float32r directly.

Pipeline exploits causality: after projecting token block tb, attention
for query i-tile it==tb runs immediately (it only needs K/V of blocks
<= tb), followed by the output projection of those tokens. This keeps
the PE dense across phases.

Device layouts (per core):
  xt  [D=2048, T=4096]   x transposed (d on partitions when tiled)
  wqt/wkt/wvt [2048,256] wq[heads,:].T etc.
  wot [256, 2048]        wo[:, head_cols].T
  partial [4096, 2048]   this core's partial output (fp32)
"""

import math
import os

import numpy as np

B, S, D, H = 2, 2048, 2048, 16
HD = 128                 # head dim
N_CORES = 8
HPC = H // N_CORES       # heads per core = 2
M = HPC * HD             # per-core feature width = 256
T = B * S                # 4096 tokens
P = 128
DK = D // P              # 16 contraction subtiles
TB = 512                 # token block == query i-tile
NTB = S // TB            # 4 per batch
IT = 512
JT = 128                 # key j-tile
SCALE = 1.0 / math.sqrt(HD)

_CACHE = {}


def _round_fp32r(a: np.ndarray) -> np.ndarray:
    """Round fp32 to fp32r (11-bit mantissa): round-to-nearest-even at bit 12."""
    u = np.ascontiguousarray(a, dtype=np.float32).view(np.uint32)
    r = (u + np.uint32(0x7FF) + ((u >> np.uint32(12)) & np.uint32(1))) & np.uint32(0xFFFFF000)
    return r.view(np.float32)


def _build_nc():
    import concourse.bacc as bacc
    import concourse.mybir as mybir
    import concourse.tile as tile
    from concourse.masks import make_identity

    f32 = mybir.dt.float32
    f32r = mybir.dt.float32r

    nc = bacc.Bacc("TRN2", target_bir_lowering=False, debug=False,
                   num_devices=N_CORES)

    xt_d = nc.dram_tensor("xt", [D, T], f32r, kind="ExternalInput").ap()
    wqt_d = nc.dram_tensor("wqt", [D, M], f32r, kind="ExternalInput").ap()
    wkt_d = nc.dram_tensor("wkt", [D, M], f32r, kind="ExternalInput").ap()
    wvt_d = nc.dram_tensor("wvt", [D, M], f32r, kind="ExternalInput").ap()
    wot_d = nc.dram_tensor("wot", [M, D], f32r, kind="ExternalInput").ap()
    out_d = nc.dram_tensor("partial", [T, D], f32, kind="ExternalOutput").ap()

    xt_tiled = xt_d.rearrange("(dk p) t -> p dk t", p=P)
    wq_tiled = wqt_d.rearrange("(dk p) m -> p dk m", p=P)
    wk_tiled = wkt_d.rearrange("(dk p) m -> p dk m", p=P)
    wv_tiled = wvt_d.rearrange("(dk p) m -> p dk m", p=P)
    wot_tiled = wot_d.rearrange("(mo p) n -> p mo n", p=P)

    HK = DK // 2  # xt streamed in two half-depth tiles per token block
    QK = DK // 4  # each half DMA'd as two quarter chunks

    with tile.TileContext(nc) as tc:
        with (
            tc.tile_pool(name="singles", bufs=1) as singles,
            tc.tile_pool(name="xt_pool", bufs=2) as xt_pool,
            tc.tile_pool(name="qt_pool", bufs=2) as qt_pool,
            tc.tile_pool(name="ot_pool", bufs=3) as ot_pool,
            tc.tile_pool(name="pt_pool", bufs=6) as pt_pool,
            tc.tile_pool(name="misc", bufs=3) as misc,
            tc.tile_pool(name="ps_big", bufs=3, space="PSUM") as ps_big_pool,
            tc.tile_pool(name="ps_o", bufs=3, space="PSUM") as ps_o_pool,
            tc.tile_pool(name="ps_den", bufs=2, space="PSUM") as ps_den_pool,
        ):
            def load_xt(t0):
                halves = []
                for hf in range(2):
                    xt_h = xt_pool.tile([P, HK, TB], f32r, name="xt_h", tag="xt")
                    for q in range(4):
                        qs = HK // 4
                        nc.sync.dma_start(
                            out=xt_h[:, q * qs:(q + 1) * qs, :],
                            in_=xt_tiled[:, hf * HK + q * qs:
                                         hf * HK + (q + 1) * qs, t0:t0 + TB])
                    halves.append(xt_h)
                return halves

            # ---- first token block's xt prefetch (before weights) ----
            first_halves = load_xt(0)

            # ---- weights (resident), DMA'd in 4 chunks each ----
            wq_sb = singles.tile([P, DK, M], f32r)
            wk_sb = singles.tile([P, DK, M], f32r)
            wv_sb = singles.tile([P, DK, M], f32r)
            wo_sb = singles.tile([P, HPC, D], f32r)
            for c in range(4):
                cs = slice(c * (DK // 4), (c + 1) * (DK // 4))
                nc.sync.dma_start(out=wq_sb[:, cs, :], in_=wq_tiled[:, cs, :])
                nc.sync.dma_start(out=wk_sb[:, cs, :], in_=wk_tiled[:, cs, :])
                nc.sync.dma_start(out=wv_sb[:, cs, :], in_=wv_tiled[:, cs, :])
                nc.sync.dma_start(out=wo_sb[:, :, c * 512:(c + 1) * 512],
                                  in_=wot_tiled[:, :, c * 512:(c + 1) * 512])

            # ---- constants ----
            onestmp = misc.tile([P, 1], f32, name="onestmp")
            nc.vector.memset(onestmp, 1.0)
            ones_col = singles.tile([P, 1], f32r)
            nc.vector.tensor_copy(ones_col, onestmp)
            ones_row = singles.tile([1, P], f32r)
            nc.vector.tensor_copy(ones_row, onestmp[0:1, 0:1].to_broadcast((1, P)))

            # HAM warmup: junk matmuls spanning the initial DMA window so the
            # PE clock is at 8/8 when real work arrives (plain fp32 — no
            # fp32r rounding requirement). Uses a memset tile so it can start
            # before the gpsimd-built identity is ready.
            warm_sb = singles.tile([P, P], f32, name="warm_sb")
            nc.vector.memset(warm_sb, 1.0)
            ps_w = ps_big_pool.tile([P, P], f32, name="ps_w", tag="big")
            for _ in range(56):
                nc.tensor.matmul(ps_w, lhsT=warm_sb, rhs=warm_sb,
                                 start=True, stop=True)

            identity = singles.tile([P, P], f32)
            make_identity(nc, identity)

            # single causal mask tile; mask for diagonal offset d is
            # maskF[:, 384-d : 896-d]  (keep iff f >= p + d)
            mtmp = misc.tile([P, IT + 384], f32, name="mtmp")
            nc.vector.memset(mtmp, 1.0)
            nc.gpsimd.affine_select(
                out=mtmp, in_=mtmp,
                compare_op=mybir.AluOpType.is_ge,
                fill=0.0, base=-384,
                pattern=[[1, IT + 384]], channel_multiplier=-1)
            maskF = singles.tile([P, IT + 384], f32r)
            nc.vector.tensor_copy(maskF, mtmp)

            def mask_for(d_off):
                return maskF[:, 384 - d_off: 896 - d_off]

            # full-batch K/V (attention for i-tile it reads blocks <= it)
            kT_sb = singles.tile([P, HPC, S], f32r)
            v_sb = singles.tile([P, S // P, M], f32r)  # [t%128, t//128, m]

            for b in range(B):
                for tb in range(NTB):
                    t0 = b * S + tb * TB
                    halves = first_halves if (b == 0 and tb == 0) else load_xt(t0)

                    def xt_at(dk):
                        return halves[dk // HK][:, dk % HK, :]

                    # ---------- Q/K projections (feature-major) ----------
                    qT_tb = qt_pool.tile([P, HPC, TB], f32r, name="qT_tb")
                    for m in range(HPC):
                        ms = slice(m * HD, (m + 1) * HD)
                        ps_q = ps_big_pool.tile([P, TB], f32, name="ps_q", tag="big")
                        for dk in range(DK):
                            nc.tensor.matmul(ps_q, lhsT=wq_sb[:, dk, ms],
                                             rhs=xt_at(dk),
                                             start=(dk == 0), stop=(dk == DK - 1))
                        nc.scalar.copy(qT_tb[:, m, :], ps_q)

                        ps_k = ps_big_pool.tile([P, TB], f32, name="ps_k", tag="big")
                        for dk in range(DK):
                            nc.tensor.matmul(ps_k, lhsT=wk_sb[:, dk, ms],
                                             rhs=xt_at(dk),
                                             start=(dk == 0), stop=(dk == DK - 1))
                        nc.vector.tensor_copy(
                            kT_sb[:, m, tb * TB:(tb + 1) * TB], ps_k)

                    # ---------- V: feature-major then PE-transpose ----------
                    for m in range(HPC):
                        ms = slice(m * HD, (m + 1) * HD)
                        ps_vt = ps_big_pool.tile([P, TB], f32, name="ps_vt", tag="big")
                        for dk in range(DK):
                            nc.tensor.matmul(ps_vt, lhsT=wv_sb[:, dk, ms],
                                             rhs=xt_at(dk),
                                             start=(dk == 0), stop=(dk == DK - 1))
                        vt_sb = misc.tile([P, TB], f32, name="vt_sb")
                        nc.vector.tensor_copy(vt_sb, ps_vt)
                        for ts4 in range(TB // P):
                            ps_t = ps_big_pool.tile([P, P], f32, name="ps_t",
                                                    tag="big")
                            nc.tensor.transpose(
                                ps_t, vt_sb[:, ts4 * P:(ts4 + 1) * P], identity)
                            nc.vector.tensor_copy(
                                v_sb[:, tb * (TB // P) + ts4, ms], ps_t)

                    # ---------- causal attention for i-tile it == tb ----------
                    # The two heads' streams are interleaved per j-tile so the
                    # PE always has an independent scores matmul in flight
                    # while ScalarE computes the other head's exp.
                    it = tb
                    njt = (it + 1) * (IT // JT)
                    oT_it = ot_pool.tile([P, HPC, IT], f32r, name="oT_it")
                    ps_os = [ps_o_pool.tile([P, IT], f32, name=f"ps_o{m}",
                                            tag="ps_o") for m in range(HPC)]
                    ps_dens = [ps_den_pool.tile([1, IT], f32, name=f"ps_den{m}",
                                                tag="ps_den")
                               for m in range(HPC)]
                    for jt in range(njt):
                        d_off = jt * JT - it * IT
                        pts = []
                        for m in range(HPC):
                            ps_s = ps_big_pool.tile([P, IT], f32, name="ps_s",
                                                    tag="big")
                            nc.tensor.matmul(
                                ps_s,
                                lhsT=kT_sb[:, m, jt * JT:(jt + 1) * JT],
                                rhs=qT_tb[:, m, :],
                                start=True, stop=True)
                            pt = pt_pool.tile([P, IT], f32r, name="pt")
                            nc.scalar.activation(
                                pt, ps_s, mybir.ActivationFunctionType.Exp,
                                scale=SCALE)
                            if d_off >= 0:
                                # diagonal block: zero entries with j > i
                                nc.vector.tensor_tensor(
                                    pt, pt, mask_for(d_off),
                                    op=mybir.AluOpType.mult)
                            pts.append(pt)
                        # both dens first (ones stationary loaded once), then AVs
                        for m in range(HPC):
                            nc.tensor.matmul(
                                ps_dens[m], lhsT=ones_col, rhs=pts[m],
                                start=(jt == 0), stop=(jt == njt - 1))
                        for m in range(HPC):
                            ms = slice(m * HD, (m + 1) * HD)
                            nc.tensor.matmul(
                                ps_os[m], lhsT=v_sb[:, jt, ms], rhs=pts[m],
                                start=(jt == 0), stop=(jt == njt - 1))
                    for m in range(HPC):
                        # normalization: broadcast den, approx-reciprocal, mult
                        den_sb = pt_pool.tile([1, IT], f32r, name="den_sb", tag="pt")
                        nc.vector.tensor_copy(den_sb, ps_dens[m])
                        ps_bd = ps_big_pool.tile([P, IT], f32, name="ps_bd", tag="big")
                        nc.tensor.matmul(ps_bd, lhsT=ones_row, rhs=den_sb,
                                         start=True, stop=True)
                        recip_bc = misc.tile([P, IT], f32, name="recip_bc")
                        nc.vector.reciprocal_approx_fast(recip_bc, ps_bd)
                        nc.vector.tensor_tensor(
                            oT_it[:, m, :], ps_os[m], recip_bc,
                            op=mybir.AluOpType.mult)

                    # ---------- output projection for these tokens ----------
                    # n-tiles in pairs so each oT stationary load covers 2 MMs
                    for tt in range(IT // P):
                        for ntp in range(D // 1024):
                            ps_ps = [ps_o_pool.tile([P, 512], f32, name=f"ps_p{q}",
                                                    tag="ps_o") for q in range(2)]
                            for mo in range(HPC):
                                for q in range(2):
                                    nt = ntp * 2 + q
                                    nc.tensor.matmul(
                                        ps_ps[q],
                                        lhsT=oT_it[:, mo, tt * P:(tt + 1) * P],
                                        rhs=wo_sb[:, mo, nt * 512:(nt + 1) * 512],
                                        start=(mo == 0), stop=(mo == HPC - 1))
                            for q in range(2):
                                nt = ntp * 2 + q
                                out_sb = misc.tile([P, 512], f32, name="out_sb")
                                nc.vector.tensor_copy(out_sb, ps_ps[q])
                                nc.sync.dma_start(
                                    out=out_d[t0 + tt * P: t0 + (tt + 1) * P,
                                              nt * 512:(nt + 1) * 512],
                                    in_=out_sb)

    nc.compile()
    return nc


def get_nc():
    if "nc" not in _CACHE:
        _CACHE["nc"] = _build_nc()
    return _CACHE["nc"]


def make_in_maps(x, wq, wk, wv, wo):
    xT = _round_fp32r(np.asarray(x, dtype=np.float32).reshape(T, D).T)
    in_maps = []
    for c in range(N_CORES):
        r = slice(c * M, (c + 1) * M)
        in_maps.append({
            "xt": xT,
            "wqt": _round_fp32r(np.asarray(wq, np.float32)[r, :].T),
            "wkt": _round_fp32r(np.asarray(wk, np.float32)[r, :].T),
            "wvt": _round_fp32r(np.asarray(wv, np.float32)[r, :].T),
            "wot": _round_fp32r(np.asarray(wo, np.float32)[:, r].T),
        })
    return in_maps


def kernel(x, wq, wk, wv, wo, bo):
    from concourse import bass_utils

    bo = np.asarray(bo, dtype=np.float32)
    nc = get_nc()
    in_maps = make_in_maps(x, wq, wk, wv, wo)
    res = bass_utils.run_bass_kernel_spmd(nc, in_maps,
                                          core_ids=list(range(N_CORES)))
    out = np.zeros((T, D), dtype=np.float32)
    for c in range(N_CORES):
        out += res.results[c]["partial"]
    out += bo[None, :]
    return out.reshape(B, S, D)


# revision 23
# speedup vs baseline: 1.1558x; 1.1558x over previous
"""Causal multi-head attention (B=2, S=2048, D=2048, H=16) on 8 trn2 cores.

Tensor-parallel over heads: each core computes 2 heads end-to-end (QKV
projection, causal attention, output projection) and returns its partial
contribution to the output projection; the host sums the 8 partials and
adds the bias (the TP all-reduce).

Matmuls run in float32r (fp32 with 11-bit mantissa, full PE rate). The
compiler requires fp32r matmul inputs to be produced rounded, so DRAM
inputs are declared float32r and pre-rounded on the host; on-chip
producers (ACT/DVE copies) write float32r directly.

Pipeline exploits causality: after projecting token block tb, attention
for query i-tile it==tb runs immediately (it only needs K/V of blocks
<= tb), followed by the output projection of those tokens. This keeps
the PE dense across phases.

Device layouts (per core):
  xt  [D=2048, T=4096]   x transposed (d on partitions when tiled)
  wqt/wkt/wvt [2048,256] wq[heads,:].T etc.
  wot [256, 2048]        wo[:, head_cols].T
  partial [4096, 2048]   this core's partial output (fp32)
"""

import math
import os

import numpy as np

B, S, D, H = 2, 2048, 2048, 16
HD = 128                 # head dim
N_CORES = 8
HPC = H // N_CORES       # heads per core = 2
M = HPC * HD             # per-core feature width = 256
T = B * S                # 4096 tokens
P = 128
DK = D // P              # 16 contraction subtiles
TB = 512                 # token block == query i-tile
NTB = S // TB            # 4 per batch
IT = 512
JT = 128                 # key j-tile
SCALE = 1.0 / math.sqrt(HD)

_CACHE = {}


def _round_fp32r(a: np.ndarray) -> np.ndarray:
    """Round fp32 to fp32r (11-bit mantissa): round-to-nearest-even at bit 12."""
    u = np.ascontiguousarray(a, dtype=np.float32).view(np.uint32)
    r = (u + np.uint32(0x7FF) + ((u >> np.uint32(12)) & np.uint32(1))) & np.uint32(0xFFFFF000)
    return r.view(np.float32)


def _build_nc():
    import concourse.bacc as bacc
    import concourse.mybir as mybir
    import concourse.tile as tile
    from concourse.masks import make_identity

    f32 = mybir.dt.float32
    f32r = mybir.dt.float32r

    nc = bacc.Bacc("TRN2", target_bir_lowering=False, debug=False,
                   num_devices=N_CORES)

    xt_d = nc.dram_tensor("xt", [D, T], f32r, kind="ExternalInput").ap()
    wqt_d = nc.dram_tensor("wqt", [D, M], f32r, kind="ExternalInput").ap()
    wkt_d = nc.dram_tensor("wkt", [D, M], f32r, kind="ExternalInput").ap()
    wvt_d = nc.dram_tensor("wvt", [D, M], f32r, kind="ExternalInput").ap()
    wot_d = nc.dram_tensor("wot", [M, D], f32r, kind="ExternalInput").ap()
    out_d = nc.dram_tensor("partial", [T, D], f32, kind="ExternalOutput").ap()

    xt_tiled = xt_d.rearrange("(dk p) t -> p dk t", p=P)
    wq_tiled = wqt_d.rearrange("(dk p) m -> p dk m", p=P)
    wk_tiled = wkt_d.rearrange("(dk p) m -> p dk m", p=P)
    wv_tiled = wvt_d.rearrange("(dk p) m -> p dk m", p=P)
    wot_tiled = wot_d.rearrange("(mo p) n -> p mo n", p=P)

    HK = DK // 2  # xt streamed in two half-depth tiles per token block
    QK = DK // 4  # each half DMA'd as two quarter chunks

    with tile.TileContext(nc) as tc:
        with (
            tc.tile_pool(name="singles", bufs=1) as singles,
            tc.tile_pool(name="xt_pool", bufs=2) as xt_pool,
            tc.tile_pool(name="qt_pool", bufs=2) as qt_pool,
            tc.tile_pool(name="ot_pool", bufs=3) as ot_pool,
            tc.tile_pool(name="pt_pool", bufs=6) as pt_pool,
            tc.tile_pool(name="misc", bufs=3) as misc,
            tc.tile_pool(name="ps_big", bufs=4, space="PSUM") as ps_big_pool,
            tc.tile_pool(name="ps_o", bufs=2, space="PSUM") as ps_o_pool,
            tc.tile_pool(name="ps_den", bufs=2, space="PSUM") as ps_den_pool,
        ):
            def load_xt(t0):
                halves = []
                for hf in range(2):
                    xt_h = xt_pool.tile([P, HK, TB], f32r, name="xt_h", tag="xt")
                    for q in range(4):
                        qs = HK // 4
                        nc.sync.dma_start(
                            out=xt_h[:, q * qs:(q + 1) * qs, :],
                            in_=xt_tiled[:, hf * HK + q * qs:
                                         hf * HK + (q + 1) * qs, t0:t0 + TB])
                    halves.append(xt_h)
                return halves

            # ---- first token block's xt prefetch (before weights) ----
            first_halves = load_xt(0)

            # ---- weights (resident), DMA'd in 4 chunks each ----
            wq_sb = singles.tile([P, DK, M], f32r)
            wk_sb = singles.tile([P, DK, M], f32r)
            wv_sb = singles.tile([P, DK, M], f32r)
            wo_sb = singles.tile([P, HPC, D], f32r)
            for c in range(4):
                cs = slice(c * (DK // 4), (c + 1) * (DK // 4))
                nc.sync.dma_start(out=wq_sb[:, cs, :], in_=wq_tiled[:, cs, :])
                nc.sync.dma_start(out=wk_sb[:, cs, :], in_=wk_tiled[:, cs, :])
                nc.sync.dma_start(out=wv_sb[:, cs, :], in_=wv_tiled[:, cs, :])
                nc.sync.dma_start(out=wo_sb[:, :, c * 512:(c + 1) * 512],
                                  in_=wot_tiled[:, :, c * 512:(c + 1) * 512])

            # ---- constants ----
            onestmp = misc.tile([P, 1], f32, name="onestmp")
            nc.vector.memset(onestmp, 1.0)
            ones_col = singles.tile([P, 1], f32r)
            nc.vector.tensor_copy(ones_col, onestmp)
            ones_row = singles.tile([1, P], f32r)
            nc.vector.tensor_copy(ones_row, onestmp[0:1, 0:1].to_broadcast((1, P)))

            # HAM warmup: junk matmuls spanning the initial DMA window so the
            # PE clock is at 8/8 when real work arrives (plain fp32 — no
            # fp32r rounding requirement). Uses a memset tile so it can start
            # before the gpsimd-built identity is ready.
            warm_sb = singles.tile([P, P], f32, name="warm_sb")
            nc.vector.memset(warm_sb, 1.0)
            ps_w = ps_big_pool.tile([P, P], f32, name="ps_w", tag="big")
            for _ in range(56):
                nc.tensor.matmul(ps_w, lhsT=warm_sb, rhs=warm_sb,
                                 start=True, stop=True)

            identity = singles.tile([P, P], f32)
            make_identity(nc, identity)

            # single causal mask tile; mask for diagonal offset d is
            # maskF[:, 384-d : 896-d]  (keep iff f >= p + d)
            mtmp = misc.tile([P, IT + 384], f32, name="mtmp")
            nc.vector.memset(mtmp, 1.0)
            nc.gpsimd.affine_select(
                out=mtmp, in_=mtmp,
                compare_op=mybir.AluOpType.is_ge,
                fill=0.0, base=-384,
                pattern=[[1, IT + 384]], channel_multiplier=-1)
            maskF = singles.tile([P, IT + 384], f32r)
            nc.vector.tensor_copy(maskF, mtmp)

            def mask_for(d_off):
                return maskF[:, 384 - d_off: 896 - d_off]

            # full-batch K/V (attention for i-tile it reads blocks <= it)
            kT_sb = singles.tile([P, HPC, S], f32r)
            v_sb = singles.tile([P, S // P, M], f32r)  # [t%128, t//128, m]

            for b in range(B):
                for tb in range(NTB):
                    t0 = b * S + tb * TB
                    halves = first_halves if (b == 0 and tb == 0) else load_xt(t0)

                    def xt_at(dk):
                        return halves[dk // HK][:, dk % HK, :]

                    # ---------- Q/K projections (feature-major) ----------
                    qT_tb = qt_pool.tile([P, HPC, TB], f32r, name="qT_tb")
                    for m in range(HPC):
                        ms = slice(m * HD, (m + 1) * HD)
                        ps_q = ps_big_pool.tile([P, TB], f32, name="ps_q", tag="big")
                        for dk in range(DK):
                            nc.tensor.matmul(ps_q, lhsT=wq_sb[:, dk, ms],
                                             rhs=xt_at(dk),
                                             start=(dk == 0), stop=(dk == DK - 1))
                        nc.scalar.copy(qT_tb[:, m, :], ps_q)

                        ps_k = ps_big_pool.tile([P, TB], f32, name="ps_k", tag="big")
                        for dk in range(DK):
                            nc.tensor.matmul(ps_k, lhsT=wk_sb[:, dk, ms],
                                             rhs=xt_at(dk),
                                             start=(dk == 0), stop=(dk == DK - 1))
                        nc.vector.tensor_copy(
                            kT_sb[:, m, tb * TB:(tb + 1) * TB], ps_k)

                    # ---------- V: feature-major then PE-transpose ----------
                    for m in range(HPC):
                        ms = slice(m * HD, (m + 1) * HD)
                        ps_vt = ps_big_pool.tile([P, TB], f32, name="ps_vt", tag="big")
                        for dk in range(DK):
                            nc.tensor.matmul(ps_vt, lhsT=wv_sb[:, dk, ms],
                                             rhs=xt_at(dk),
                                             start=(dk == 0), stop=(dk == DK - 1))
                        vt_sb = misc.tile([P, TB], f32, name="vt_sb")
                        nc.vector.tensor_copy(vt_sb, ps_vt)
                        for ts4 in range(TB // P):
                            ps_t = ps_big_pool.tile([P, P], f32, name="ps_t",
                                                    tag="big")
                            nc.tensor.transpose(
                                ps_t, vt_sb[:, ts4 * P:(ts4 + 1) * P], identity)
                            nc.vector.tensor_copy(
                                v_sb[:, tb * (TB // P) + ts4, ms], ps_t)

                    # ---------- causal attention for i-tile it == tb ----------
                    # The two heads' streams are interleaved per j-tile so the
                    # PE always has an independent scores matmul in flight
                    # while ScalarE computes the other head's exp.
                    it = tb
                    njt = (it + 1) * (IT // JT)
                    oT_it = ot_pool.tile([P, HPC, IT], f32r, name="oT_it")
                    ps_os = [ps_o_pool.tile([P, IT], f32, name=f"ps_o{m}",
                                            tag="ps_o") for m in range(HPC)]
                    ps_dens = [ps_den_pool.tile([1, IT], f32, name=f"ps_den{m}",
                                                tag="ps_den")
                               for m in range(HPC)]
                    for jt in range(njt):
                        d_off = jt * JT - it * IT
                        pts = []
                        for m in range(HPC):
                            ps_s = ps_big_pool.tile([P, IT], f32, name="ps_s",
                                                    tag="big")
                            nc.tensor.matmul(
                                ps_s,
                                lhsT=kT_sb[:, m, jt * JT:(jt + 1) * JT],
                                rhs=qT_tb[:, m, :],
                                start=True, stop=True)
                            pt = pt_pool.tile([P, IT], f32r, name="pt")
                            nc.scalar.activation(
                                pt, ps_s, mybir.ActivationFunctionType.Exp,
                                scale=SCALE)
                            if d_off >= 0:
                                # diagonal block: zero entries with j > i
                                nc.vector.tensor_tensor(
                                    pt, pt, mask_for(d_off),
                                    op=mybir.AluOpType.mult)
                            pts.append(pt)
                        # both dens first (ones stationary loaded once), then AVs
                        for m in range(HPC):
                            nc.tensor.matmul(
                                ps_dens[m], lhsT=ones_col, rhs=pts[m],
                                start=(jt == 0), stop=(jt == njt - 1))
                        for m in range(HPC):
                            ms = slice(m * HD, (m + 1) * HD)
                            nc.tensor.matmul(
                                ps_os[m], lhsT=v_sb[:, jt, ms], rhs=pts[m],
                                start=(jt == 0), stop=(jt == njt - 1))
                    for m in range(HPC):
                        # normalization: broadcast den, approx-reciprocal, mult
                        den_sb = pt_pool.tile([1, IT], f32r, name="den_sb", tag="pt")
                        nc.vector.tensor_copy(den_sb, ps_dens[m])
                        ps_bd = ps_big_pool.tile([P, IT], f32, name="ps_bd", tag="big")
                        nc.tensor.matmul(ps_bd, lhsT=ones_row, rhs=den_sb,
                                         start=True, stop=True)
                        recip_bc = misc.tile([P, IT], f32, name="recip_bc")
                        nc.vector.reciprocal_approx_fast(recip_bc, ps_bd)
                        nc.vector.tensor_tensor(
                            oT_it[:, m, :], ps_os[m], recip_bc,
                            op=mybir.AluOpType.mult)

                    # ---------- output projection for these tokens ----------
                    # n-tiles in pairs so each oT stationary load covers 2 MMs
                    for tt in range(IT // P):
                        for ntp in range(D // 1024):
                            ps_ps = [ps_o_pool.tile([P, 512], f32, name=f"ps_p{q}",
                                                    tag="ps_o") for q in range(2)]
                            for mo in range(HPC):
                                for q in range(2):
                                    nt = ntp * 2 + q
                                    nc.tensor.matmul(
                                        ps_ps[q],
                                        lhsT=oT_it[:, mo, tt * P:(tt + 1) * P],
                                        rhs=wo_sb[:, mo, nt * 512:(nt + 1) * 512],
                                        start=(mo == 0), stop=(mo == HPC - 1))
                            for q in range(2):
                                nt = ntp * 2 + q
                                out_sb = misc.tile([P, 512], f32, name="out_sb")
                                nc.vector.tensor_copy(out_sb, ps_ps[q])
                                nc.sync.dma_start(
                                    out=out_d[t0 + tt * P: t0 + (tt + 1) * P,
                                              nt * 512:(nt + 1) * 512],
                                    in_=out_sb)

    nc.compile()
    return nc


def get_nc():
    if "nc" not in _CACHE:
        _CACHE["nc"] = _build_nc()
    return _CACHE["nc"]


def make_in_maps(x, wq, wk, wv, wo):
    xT = _round_fp32r(np.asarray(x, dtype=np.float32).reshape(T, D).T)
    in_maps = []
    for c in range(N_CORES):
        r = slice(c * M, (c + 1) * M)
        in_maps.append({
            "xt": xT,
            "wqt": _round_fp32r(np.asarray(wq, np.float32)[r, :].T),
            "wkt": _round_fp32r(np.asarray(wk, np.float32)[r, :].T),
            "wvt": _round_fp32r(np.asarray(wv, np.float32)[r, :].T),
            "wot": _round_fp32r(np.asarray(wo, np.float32)[:, r].T),
        })
    return in_maps


def kernel(x, wq, wk, wv, wo, bo):
    from concourse import bass_utils

    bo = np.asarray(bo, dtype=np.float32)
    nc = get_nc()
    in_maps = make_in_maps(x, wq, wk, wv, wo)
    res = bass_utils.run_bass_kernel_spmd(nc, in_maps,
                                          core_ids=list(range(N_CORES)))
    out = np.zeros((T, D), dtype=np.float32)
    for c in range(N_CORES):
        out += res.results[c]["partial"]
    out += bo[None, :]
    return out.reshape(B, S, D)


# revision 24
# speedup vs baseline: 1.2074x; 1.0447x over previous
"""Causal multi-head attention (B=2, S=2048, D=2048, H=16) on 8 trn2 cores.

Tensor-parallel over heads: each core computes 2 heads end-to-end (QKV
projection, causal attention, output projection) and returns its partial
contribution to the output projection; the host sums the 8 partials and
adds the bias (the TP all-reduce).

Matmuls run in float32r (fp32 with 11-bit mantissa, full PE rate). The
compiler requires fp32r matmul inputs to be produced rounded, so DRAM
inputs are declared float32r and pre-rounded on the host; on-chip
producers (ACT/DVE copies) write float32r directly.

Pipeline exploits causality: after projecting token block tb, attention
for query i-tile it==tb runs immediately (it only needs K/V of blocks
<= tb), followed by the output projection of those tokens. This keeps
the PE dense across phases.

Device layouts (per core):
  xt  [D=2048, T=4096]   x transposed (d on partitions when tiled)
  wqt/wkt/wvt [2048,256] wq[heads,:].T etc.
  wot [256, 2048]        wo[:, head_cols].T
  partial [4096, 2048]   this core's partial output (fp32)
"""

import math
import os

import numpy as np

B, S, D, H = 2, 2048, 2048, 16
HD = 128                 # head dim
N_CORES = 8
HPC = H // N_CORES       # heads per core = 2
M = HPC * HD             # per-core feature width = 256
T = B * S                # 4096 tokens
P = 128
DK = D // P              # 16 contraction subtiles
TB = 512                 # token block == query i-tile
NTB = S // TB            # 4 per batch
IT = 512
JT = 128                 # key j-tile
SCALE = 1.0 / math.sqrt(HD)

_CACHE = {}


def _round_fp32r(a: np.ndarray) -> np.ndarray:
    """Round fp32 to fp32r (11-bit mantissa): round-to-nearest-even at bit 12."""
    u = np.ascontiguousarray(a, dtype=np.float32).view(np.uint32)
    r = (u + np.uint32(0x7FF) + ((u >> np.uint32(12)) & np.uint32(1))) & np.uint32(0xFFFFF000)
    return r.view(np.float32)


def _build_nc():
    import concourse.bacc as bacc
    import concourse.mybir as mybir
    import concourse.tile as tile
    from concourse.masks import make_identity

    f32 = mybir.dt.float32
    f32r = mybir.dt.float32r

    nc = bacc.Bacc("TRN2", target_bir_lowering=False, debug=False,
                   num_devices=N_CORES)

    xt_d = nc.dram_tensor("xt", [D, T], f32r, kind="ExternalInput").ap()
    wqt_d = nc.dram_tensor("wqt", [D, M], f32r, kind="ExternalInput").ap()
    wkt_d = nc.dram_tensor("wkt", [D, M], f32r, kind="ExternalInput").ap()
    wvt_d = nc.dram_tensor("wvt", [D, M], f32r, kind="ExternalInput").ap()
    wot_d = nc.dram_tensor("wot", [M, D], f32r, kind="ExternalInput").ap()
    out_d = nc.dram_tensor("partial", [T, D], f32, kind="ExternalOutput").ap()

    xt_tiled = xt_d.rearrange("(dk p) t -> p dk t", p=P)
    wq_tiled = wqt_d.rearrange("(dk p) m -> p dk m", p=P)
    wk_tiled = wkt_d.rearrange("(dk p) m -> p dk m", p=P)
    wv_tiled = wvt_d.rearrange("(dk p) m -> p dk m", p=P)
    wot_tiled = wot_d.rearrange("(mo p) n -> p mo n", p=P)

    HK = DK // 2  # xt streamed in two half-depth tiles per token block
    QK = DK // 4  # each half DMA'd as two quarter chunks

    with tile.TileContext(nc) as tc:
        with (
            tc.tile_pool(name="singles", bufs=1) as singles,
            tc.tile_pool(name="xt_pool", bufs=2) as xt_pool,
            tc.tile_pool(name="qt_pool", bufs=2) as qt_pool,
            tc.tile_pool(name="ot_pool", bufs=3) as ot_pool,
            tc.tile_pool(name="pt_pool", bufs=6) as pt_pool,
            tc.tile_pool(name="misc", bufs=3) as misc,
            tc.tile_pool(name="ps_big", bufs=4, space="PSUM") as ps_big_pool,
            tc.tile_pool(name="ps_o", bufs=2, space="PSUM") as ps_o_pool,
            tc.tile_pool(name="ps_den", bufs=2, space="PSUM") as ps_den_pool,
        ):
            def load_xt(t0):
                halves = []
                for hf in range(2):
                    xt_h = xt_pool.tile([P, HK, TB], f32r, name="xt_h", tag="xt")
                    for q in range(2):
                        nc.sync.dma_start(
                            out=xt_h[:, q * QK:(q + 1) * QK, :],
                            in_=xt_tiled[:, hf * HK + q * QK:
                                         hf * HK + (q + 1) * QK, t0:t0 + TB])
                    halves.append(xt_h)
                return halves

            # ---- first token block's xt prefetch (before weights) ----
            first_halves = load_xt(0)

            # ---- weights (resident), DMA'd in 4 chunks each ----
            wq_sb = singles.tile([P, DK, M], f32r)
            wk_sb = singles.tile([P, DK, M], f32r)
            wv_sb = singles.tile([P, DK, M], f32r)
            wo_sb = singles.tile([P, HPC, D], f32r)
            for c in range(4):
                cs = slice(c * (DK // 4), (c + 1) * (DK // 4))
                nc.sync.dma_start(out=wq_sb[:, cs, :], in_=wq_tiled[:, cs, :])
                nc.sync.dma_start(out=wk_sb[:, cs, :], in_=wk_tiled[:, cs, :])
                nc.sync.dma_start(out=wv_sb[:, cs, :], in_=wv_tiled[:, cs, :])
                nc.sync.dma_start(out=wo_sb[:, :, c * 512:(c + 1) * 512],
                                  in_=wot_tiled[:, :, c * 512:(c + 1) * 512])

            # ---- constants ----
            onestmp = misc.tile([P, 1], f32, name="onestmp")
            nc.vector.memset(onestmp, 1.0)
            ones_col = singles.tile([P, 1], f32r)
            nc.vector.tensor_copy(ones_col, onestmp)
            ones_row = singles.tile([1, P], f32r)
            nc.vector.tensor_copy(ones_row, onestmp[0:1, 0:1].to_broadcast((1, P)))

            identity = singles.tile([P, P], f32)
            make_identity(nc, identity)

            # HAM warmup: junk matmuls spanning the initial DMA window so the
            # PE clock is at 8/8 when real work arrives (plain fp32 — no
            # fp32r rounding requirement).
            ps_w = ps_big_pool.tile([P, P], f32, name="ps_w", tag="big")
            for _ in range(56):
                nc.tensor.matmul(ps_w, lhsT=identity, rhs=identity,
                                 start=True, stop=True)

            # single causal mask tile; mask for diagonal offset d is
            # maskF[:, 384-d : 896-d]  (keep iff f >= p + d)
            mtmp = misc.tile([P, IT + 384], f32, name="mtmp")
            nc.vector.memset(mtmp, 1.0)
            nc.gpsimd.affine_select(
                out=mtmp, in_=mtmp,
                compare_op=mybir.AluOpType.is_ge,
                fill=0.0, base=-384,
                pattern=[[1, IT + 384]], channel_multiplier=-1)
            maskF = singles.tile([P, IT + 384], f32r)
            nc.vector.tensor_copy(maskF, mtmp)

            def mask_for(d_off):
                return maskF[:, 384 - d_off: 896 - d_off]

            # full-batch K/V (attention for i-tile it reads blocks <= it)
            kT_sb = singles.tile([P, HPC, S], f32r)
            v_sb = singles.tile([P, S // P, M], f32r)  # [t%128, t//128, m]

            for b in range(B):
                for tb in range(NTB):
                    t0 = b * S + tb * TB
                    halves = first_halves if (b == 0 and tb == 0) else load_xt(t0)

                    def xt_at(dk):
                        return halves[dk // HK][:, dk % HK, :]

                    # ---------- Q/K projections (feature-major) ----------
                    qT_tb = qt_pool.tile([P, HPC, TB], f32r, name="qT_tb")
                    for m in range(HPC):
                        ms = slice(m * HD, (m + 1) * HD)
                        ps_q = ps_big_pool.tile([P, TB], f32, name="ps_q", tag="big")
                        for dk in range(DK):
                            nc.tensor.matmul(ps_q, lhsT=wq_sb[:, dk, ms],
                                             rhs=xt_at(dk),
                                             start=(dk == 0), stop=(dk == DK - 1))
                        nc.scalar.copy(qT_tb[:, m, :], ps_q)

                        ps_k = ps_big_pool.tile([P, TB], f32, name="ps_k", tag="big")
                        for dk in range(DK):
                            nc.tensor.matmul(ps_k, lhsT=wk_sb[:, dk, ms],
                                             rhs=xt_at(dk),
                                             start=(dk == 0), stop=(dk == DK - 1))
                        nc.vector.tensor_copy(
                            kT_sb[:, m, tb * TB:(tb + 1) * TB], ps_k)

                    # ---------- V: feature-major then PE-transpose ----------
                    for m in range(HPC):
                        ms = slice(m * HD, (m + 1) * HD)
                        ps_vt = ps_big_pool.tile([P, TB], f32, name="ps_vt", tag="big")
                        for dk in range(DK):
                            nc.tensor.matmul(ps_vt, lhsT=wv_sb[:, dk, ms],
                                             rhs=xt_at(dk),
                                             start=(dk == 0), stop=(dk == DK - 1))
                        vt_sb = misc.tile([P, TB], f32, name="vt_sb")
                        nc.vector.tensor_copy(vt_sb, ps_vt)
                        for ts4 in range(TB // P):
                            ps_t = ps_big_pool.tile([P, P], f32, name="ps_t",
                                                    tag="big")
                            nc.tensor.transpose(
                                ps_t, vt_sb[:, ts4 * P:(ts4 + 1) * P], identity)
                            nc.vector.tensor_copy(
                                v_sb[:, tb * (TB // P) + ts4, ms], ps_t)

                    # ---------- causal attention for i-tile it == tb ----------
                    # The two heads' streams are interleaved per j-tile so the
                    # PE always has an independent scores matmul in flight
                    # while ScalarE computes the other head's exp.
                    it = tb
                    njt = (it + 1) * (IT // JT)
                    oT_it = ot_pool.tile([P, HPC, IT], f32r, name="oT_it")
                    ps_os = [ps_o_pool.tile([P, IT], f32, name=f"ps_o{m}",
                                            tag="ps_o") for m in range(HPC)]
                    ps_dens = [ps_den_pool.tile([1, IT], f32, name=f"ps_den{m}",
                                                tag="ps_den")
                               for m in range(HPC)]
                    for jt in range(njt):
                        d_off = jt * JT - it * IT
                        pts = []
                        for m in range(HPC):
                            ps_s = ps_big_pool.tile([P, IT], f32, name="ps_s",
                                                    tag="big")
                            nc.tensor.matmul(
                                ps_s,
                                lhsT=kT_sb[:, m, jt * JT:(jt + 1) * JT],
                                rhs=qT_tb[:, m, :],
                                start=True, stop=True)
                            pt = pt_pool.tile([P, IT], f32r, name="pt")
                            nc.scalar.activation(
                                pt, ps_s, mybir.ActivationFunctionType.Exp,
                                scale=SCALE)
                            if d_off >= 0:
                                # diagonal block: zero entries with j > i
                                nc.vector.tensor_tensor(
                                    pt, pt, mask_for(d_off),
                                    op=mybir.AluOpType.mult)
                            pts.append(pt)
                        # both dens first (ones stationary loaded once), then AVs
                        for m in range(HPC):
                            nc.tensor.matmul(
                                ps_dens[m], lhsT=ones_col, rhs=pts[m],
                                start=(jt == 0), stop=(jt == njt - 1))
                        for m in range(HPC):
                            ms = slice(m * HD, (m + 1) * HD)
                            nc.tensor.matmul(
                                ps_os[m], lhsT=v_sb[:, jt, ms], rhs=pts[m],
                                start=(jt == 0), stop=(jt == njt - 1))
                    for m in range(HPC):
                        # normalization: broadcast den, approx-reciprocal, mult
                        den_sb = pt_pool.tile([1, IT], f32r, name="den_sb", tag="pt")
                        nc.vector.tensor_copy(den_sb, ps_dens[m])
                        ps_bd = ps_big_pool.tile([P, IT], f32, name="ps_bd", tag="big")
                        nc.tensor.matmul(ps_bd, lhsT=ones_row, rhs=den_sb,
                                         start=True, stop=True)
                        recip_bc = misc.tile([P, IT], f32, name="recip_bc")
                        nc.vector.reciprocal_approx_fast(recip_bc, ps_bd)
                        nc.vector.tensor_tensor(
                            oT_it[:, m, :], ps_os[m], recip_bc,
                            op=mybir.AluOpType.mult)

                    # ---------- output projection for these tokens ----------
                    # n-tiles in pairs so each oT stationary load covers 2 MMs
                    for tt in range(IT // P):
                        for ntp in range(D // 1024):
                            ps_ps = [ps_o_pool.tile([P, 512], f32, name=f"ps_p{q}",
                                                    tag="ps_o") for q in range(2)]
                            for mo in range(HPC):
                                for q in range(2):
                                    nt = ntp * 2 + q
                                    nc.tensor.matmul(
                                        ps_ps[q],
                                        lhsT=oT_it[:, mo, tt * P:(tt + 1) * P],
                                        rhs=wo_sb[:, mo, nt * 512:(nt + 1) * 512],
                                        start=(mo == 0), stop=(mo == HPC - 1))
                            for q in range(2):
                                nt = ntp * 2 + q
                                out_sb = misc.tile([P, 512], f32, name="out_sb")
                                nc.vector.tensor_copy(out_sb, ps_ps[q])
                                nc.sync.dma_start(
                                    out=out_d[t0 + tt * P: t0 + (tt + 1) * P,
                                              nt * 512:(nt + 1) * 512],
                                    in_=out_sb)

    nc.compile()
    return nc


def get_nc():
    if "nc" not in _CACHE:
        _CACHE["nc"] = _build_nc()
    return _CACHE["nc"]


def make_in_maps(x, wq, wk, wv, wo):
    xT = _round_fp32r(np.asarray(x, dtype=np.float32).reshape(T, D).T)
    in_maps = []
    for c in range(N_CORES):
        r = slice(c * M, (c + 1) * M)
        in_maps.append({
            "xt": xT,
            "wqt": _round_fp32r(np.asarray(wq, np.float32)[r, :].T),
            "wkt": _round_fp32r(np.asarray(wk, np.float32)[r, :].T),
            "wvt": _round_fp32r(np.asarray(wv, np.float32)[r, :].T),
            "wot": _round_fp32r(np.asarray(wo, np.float32)[:, r].T),
        })
    return in_maps


def kernel(x, wq, wk, wv, wo, bo):
    from concourse import bass_utils

    bo = np.asarray(bo, dtype=np.float32)
    nc = get_nc()
    in_maps = make_in_maps(x, wq, wk, wv, wo)
    res = bass_utils.run_bass_kernel_spmd(nc, in_maps,
                                          core_ids=list(range(N_CORES)))
    out = np.zeros((T, D), dtype=np.float32)
    for c in range(N_CORES):
        out += res.results[c]["partial"]
    out += bo[None, :]
    return out.reshape(B, S, D)
